# revision 19
# baseline (speedup 1.0000x reference)
"""GNN message-passing kernel for Trainium2 (8 NeuronCores).

Reference computation:
    out[b,i,f] = X[b,0,i,i,f] + sum_{k=1..3} sum_j A[b,i,j] * X[b,k,i,j,f]

Sharding: 8 cores = (batch b in 0..3) x (i-half h in 0..1); each core owns
a (b, 128-row i-slab) of the output. Hop 0 only contributes its diagonal,
so only X[b,1:4] (3/4 of X) plus the hop-0 diagonal rows are ever sent to
the device: ~25 MB per core -> ~60 us of HBM reads at the ~420 GB/s
per-core ceiling. The kernel is DMA-bound; all compute hides under it.

Per-core device kernel, per 32-j chunk (8 KB contiguous runs per
partition; 16-j / 8-j tail chunks shrink the post-DMA tail):
  - Hop sum: slab 0 (first 8 j) on DVE as two in-place fp32 adds; slabs
    1..3 on the TensorEngine as identity-stationary FLOAT32R matmuls
    (1-pass fp32, measured 396/631 ns per 512-col MM warm/cold vs 853+
    for fp32) accumulating the 3 hops into PSUM. The split keeps BOTH
    engines below the DMA cadence even when the HAM throttles PE to
    1.2 GHz (it re-throttles on any >3.4 us idle gap, and NTFF shows it
    cold most of the time regardless of occupancy).
  - DVE: broadcast-AP multiplies by A (contiguous; strided reads cost
    1.7x, strided MOVING matmul APs 2x -- keep everything contiguous),
    a fold for 32-j chunks, and a running add into a persistent
    [128, 16*F] j-slot accumulator; j slots collapse at the end with
    fold-halves adds (high slots fold early, under the tail DMAs).
  - Hop-0 diagonal is added at the end.

fp32r rounds the moving operand (identity weights exact): rel err ~1e-4,
well inside the 2e-2 budget.
"""

import sys

if "/opt/trn_rl_repo" not in sys.path:
    sys.path.insert(0, "/opt/trn_rl_repo")

import numpy as np

import concourse.bacc as bacc
import concourse.bass as bass
import concourse.mybir as mybir
from concourse.bass_utils import run_bass_kernel_spmd
from concourse.tile import TileContext

BATCH, KP1, N, F = 4, 4, 256, 64
NH = N // 2          # 128 rows of output per core (partition dim)
CJS = [32] * 6 + [16, 16] + [8] * 4
JSLOTS = 16
FP32 = mybir.dt.float32
FP32R = mybir.dt.float32r

_CACHE = {}


def _build_nc():
    if "nc" in _CACHE:
        return _CACHE["nc"]
    nc = bacc.Bacc("TRN2", target_bir_lowering=False, debug=False, num_devices=8)
    xk = nc.dram_tensor("xk", [3, NH, N, F], FP32R, kind="ExternalInput").ap()
    a = nc.dram_tensor("a", [NH, N], FP32, kind="ExternalInput").ap()
    d = nc.dram_tensor("d", [NH, F], FP32, kind="ExternalInput").ap()
    eye = nc.dram_tensor("eye", [128, 128], FP32R, kind="ExternalInput").ap()
    out = nc.dram_tensor("out", [NH, F], FP32, kind="ExternalOutput").ap()

    with TileContext(nc) as tc:
        with (
            tc.tile_pool(name="const", bufs=1) as cpool,
            tc.tile_pool(name="xs", bufs=4) as xpool,
            tc.tile_pool(name="pr", bufs=2) as prpool,
            tc.tile_pool(name="hs", bufs=2) as hspool,
            tc.tile_pool(name="sm", bufs=2) as smpool,
            tc.tile_pool(name="ac", bufs=1) as acpool,
            tc.tile_pool(name="ps", bufs=2, space="PSUM") as pspool,
            tc.tile_pool(name="wm", bufs=1, space="PSUM") as wmpool,
        ):
            eye_sb = cpool.tile([128, 128], FP32R)
            nc.sync.dma_start(out=eye_sb[:, :], in_=eye[:, :])
            a_sb = cpool.tile([128, N], FP32)
            nc.sync.dma_start(out=a_sb[:, :], in_=a[:, :])
            d_sb = cpool.tile([128, F], FP32)
            nc.sync.dma_start(out=d_sb[:, :], in_=d[:, :])

            # PE warm-up burst during the DMA ramp (output unused).
            warm = wmpool.tile([128, 512], FP32, name="warm", tag="warm")
            for _ in range(10):
                nc.tensor.matmul(
                    warm[:, 0:128], eye_sb[:, :], eye_sb[:, :],
                    start=True, stop=True,
                )

            wsum = acpool.tile([128, JSLOTS * F], FP32)
            a_step = a_sb.ap[0][0]

            j0 = 0
            for c, CJ in enumerate(CJS):
                CJF = CJ * F
                xt = xpool.tile([128, 3 * CJF], FP32R, name="xt", tag="xt")
                for k in range(3):
                    src = bass.AP(
                        xk.tensor,
                        k * NH * N * F + j0 * F,
                        [[N * F, 128], [1, CJF]],
                    )
                    nc.sync.dma_start(
                        out=xt[:, k * CJF : (k + 1) * CJF], in_=src
                    )

                nslab = CJF // 512
                offl = nslab > 2   # 32-j chunks: slab 0 on DVE, rest on PE
                if offl:
                    # Separate fp32 output tile: writing into xt itself
                    # trips the BIR verifier (fp32r-consumed location
                    # written by a non-fp32r-rounding producer).
                    hsum = hspool.tile([128, 512], FP32, name="hs", tag="hs")
                    nc.vector.tensor_add(
                        hsum[:, :],
                        xt[:, 0:512].bitcast(FP32),
                        xt[:, CJF : CJF + 512].bitcast(FP32),
                    )
                    nc.vector.tensor_add(
                        hsum[:, :],
                        hsum[:, :],
                        xt[:, 2 * CJF : 2 * CJF + 512].bitcast(FP32),
                    )
                pe_slabs = list(range(1, nslab)) if offl else list(range(nslab))
                ps = pspool.tile(
                    [128, len(pe_slabs) * 512], FP32, name="ps", tag="ps"
                )
                for si, s in enumerate(pe_slabs):
                    for k in range(3):
                        nc.tensor.matmul(
                            ps[:, si * 512 : (si + 1) * 512],
                            eye_sb[:, :],
                            xt[:, k * CJF + s * 512 : k * CJF + (s + 1) * 512],
                            start=(k == 0),
                            stop=(k == 2),
                        )

                # prod[i, j*F+f] = hopsum[i, j*F+f] * a_sb[i, j0+j]
                prod = prpool.tile([128, CJF], FP32, name="prod", tag="prod")
                pr_step = prod.ap[0][0]
                ps_step = ps.ap[0][0]
                if offl:
                    po_a = bass.AP(
                        prod.tensor, 0, [[pr_step, 128], [F, 8], [1, F]]
                    )
                    in1a = bass.AP(
                        a_sb.tensor, j0, [[a_step, 128], [1, 8], [0, F]]
                    )
                    nc.vector.tensor_mul(po_a, hsum[:, :], in1a)
                    po_b = bass.AP(
                        prod.tensor, 512, [[pr_step, 128], [F, CJ - 8], [1, F]]
                    )
                    in0b = bass.AP(
                        ps.tensor, 0, [[ps_step, 128], [F, CJ - 8], [1, F]]
                    )
                    in1b = bass.AP(
                        a_sb.tensor, j0 + 8, [[a_step, 128], [1, CJ - 8], [0, F]]
                    )
                    nc.vector.tensor_mul(po_b, in0b, in1b)
                else:
                    po = bass.AP(
                        prod.tensor, 0, [[pr_step, 128], [F, CJ], [1, F]]
                    )
                    in0 = bass.AP(
                        ps.tensor, 0, [[ps_step, 128], [F, CJ], [1, F]]
                    )
                    in1 = bass.AP(
                        a_sb.tensor, j0, [[a_step, 128], [1, CJ], [0, F]]
                    )
                    nc.vector.tensor_mul(po, in0, in1)

                # Accumulate into the 16 j-slot accumulator.
                if c == 0:
                    # CJ=32: fold the two 1024-el halves straight into wsum.
                    nc.vector.tensor_add(
                        wsum[:, :], prod[:, :1024], prod[:, 1024:2048]
                    )
                else:
                    if CJF == 2048:
                        nc.vector.tensor_add(
                            prod[:, :1024], prod[:, :1024], prod[:, 1024:2048]
                        )
                    w = min(CJF, 1024)
                    nc.vector.tensor_add(
                        wsum[:, :w], wsum[:, :w], prod[:, :w]
                    )
                j0 += CJ
                if c == 7:
                    # Slots 8..15 are final once the 16-j chunk is in (the
                    # 8-j tail chunks only touch 0..7); fold them now,
                    # overlapped with the tail DMAs.
                    nc.vector.tensor_add(
                        wsum[:, : 8 * F],
                        wsum[:, : 8 * F],
                        wsum[:, 8 * F : 16 * F],
                    )

            # Collapse the remaining 8 j slots, then add hop-0 diagonal.
            h = JSLOTS // 4
            while h >= 1:
                nc.vector.tensor_add(
                    wsum[:, : h * F],
                    wsum[:, : h * F],
                    wsum[:, h * F : 2 * h * F],
                )
                h //= 2
            outt = smpool.tile([128, F], FP32, name="outt", tag="outt")
            nc.vector.tensor_add(outt[:, :], wsum[:, :F], d_sb[:, :])
            nc.sync.dma_start(out=out[:, :], in_=outt[:, :])

    nc.compile()
    _CACHE["nc"] = nc
    return nc


def _make_in_maps(A, X):
    idx = np.arange(NH)
    in_maps = []
    for c in range(8):
        b, h = c // 2, c % 2
        lo = h * NH
        xk = np.ascontiguousarray(X[b, 1:4, lo : lo + NH])
        av = np.ascontiguousarray(A[b, lo : lo + NH, :])
        dv = np.ascontiguousarray(X[b, 0, lo + idx, lo + idx, :])
        in_maps.append(
            {"xk": xk, "a": av, "d": dv, "eye": np.eye(128, dtype=np.float32)}
        )
    return in_maps


def run(A, X, trace=False, **kw):
    nc = _build_nc()
    in_maps = _make_in_maps(A, X)
    res = run_bass_kernel_spmd(
        nc, in_maps, core_ids=list(range(8)), trace=trace, **kw
    )
    out = np.empty((BATCH, N, F), dtype=np.float32)
    for c in range(8):
        b, h = c // 2, c % 2
        out[b, h * NH : (h + 1) * NH] = res.results[c]["out"]
    return out, res


def kernel(A, X):
    A = np.asarray(A, dtype=np.float32)
    X = np.asarray(X, dtype=np.float32)
    out, _ = run(A, X, trace=False)
    return out


# revision 27
# speedup vs baseline: 1.1257x; 1.1257x over previous
"""GNN message-passing kernel for Trainium2 (8 NeuronCores).

Reference computation:
    out[b,i,f] = X[b,0,i,i,f] + sum_{k=1..3} sum_j A[b,i,j] * X[b,k,i,j,f]

Sharding: 8 cores = (batch b in 0..3) x (i-half h in 0..1); each core owns
a (b, 128-row i-slab) of the output. Hop 0 only contributes its diagonal,
so only X[b,1:4] (3/4 of X) plus the hop-0 diagonal rows are ever sent to
the device: ~25 MB per core -> ~60 us of HBM reads at the ~420 GB/s
per-core ceiling. The kernel is DMA-bound; all compute hides under it.

Per-core device kernel, per 32-j chunk (8 KB contiguous runs per
partition; 16-j / 8-j tail chunks shrink the post-DMA tail):
  - Hop sum: slab 0 (first 8 j) on DVE as two in-place fp32 adds; slabs
    1..3 on the TensorEngine as identity-stationary FLOAT32R matmuls
    (1-pass fp32, measured 396/631 ns per 512-col MM warm/cold vs 853+
    for fp32) accumulating the 3 hops into PSUM. The split keeps BOTH
    engines below the DMA cadence even when the HAM throttles PE to
    1.2 GHz (it re-throttles on any >3.4 us idle gap, and NTFF shows it
    cold most of the time regardless of occupancy).
  - DVE: broadcast-AP multiplies by A (contiguous; strided reads cost
    1.7x, strided MOVING matmul APs 2x -- keep everything contiguous),
    a fold for 32-j chunks, and a running add into a persistent
    [128, 16*F] j-slot accumulator; j slots collapse at the end with
    fold-halves adds (high slots fold early, under the tail DMAs).
  - Hop-0 diagonal is added at the end.

fp32r rounds the moving operand (identity weights exact): rel err ~1e-4,
well inside the 2e-2 budget.
"""

import sys

if "/opt/trn_rl_repo" not in sys.path:
    sys.path.insert(0, "/opt/trn_rl_repo")

import numpy as np

import concourse.bacc as bacc
import concourse.bass as bass
import concourse.mybir as mybir
from concourse.bass_utils import run_bass_kernel_spmd
from concourse.tile import TileContext

BATCH, KP1, N, F = 4, 4, 256, 64
NH = N // 2          # 128 rows of output per core (partition dim)
CJS = [32] * 7 + [16, 8, 8]
JSLOTS = 16
FP32 = mybir.dt.float32
FP32R = mybir.dt.float32r

_CACHE = {}


def _build_nc():
    if "nc" in _CACHE:
        return _CACHE["nc"]
    nc = bacc.Bacc("TRN2", target_bir_lowering=False, debug=False, num_devices=8)
    xk = nc.dram_tensor("xk", [3, NH, N, F], FP32R, kind="ExternalInput").ap()
    a = nc.dram_tensor("a", [NH, N], FP32, kind="ExternalInput").ap()
    d = nc.dram_tensor("d", [NH, F], FP32, kind="ExternalInput").ap()
    eye = nc.dram_tensor("eye", [128, 128], FP32R, kind="ExternalInput").ap()
    out = nc.dram_tensor("out", [NH, F], FP32, kind="ExternalOutput").ap()

    with TileContext(nc) as tc:
        with (
            tc.tile_pool(name="const", bufs=1) as cpool,
            tc.tile_pool(name="xs", bufs=5) as xpool,
            tc.tile_pool(name="pr", bufs=2) as prpool,
            tc.tile_pool(name="hs", bufs=2) as hspool,
            tc.tile_pool(name="sm", bufs=2) as smpool,
            tc.tile_pool(name="ac", bufs=1) as acpool,
            tc.tile_pool(name="ps", bufs=2, space="PSUM") as pspool,
            tc.tile_pool(name="wm", bufs=1, space="PSUM") as wmpool,
        ):
            # Prefetch the first two chunks' X streams BEFORE the tiny
            # const DMAs so the big HBM transfers start immediately.
            xt_pre = []
            pj0 = 0
            for pc in range(2):
                pCJF = CJS[pc] * F
                xtp = xpool.tile([128, 3 * pCJF], FP32R, name="xt", tag="xt")
                for k in range(3):
                    src = bass.AP(
                        xk.tensor,
                        k * NH * N * F + pj0 * F,
                        [[N * F, 128], [1, pCJF]],
                    )
                    nc.sync.dma_start(
                        out=xtp[:, k * pCJF : (k + 1) * pCJF], in_=src
                    )
                xt_pre.append(xtp)
                pj0 += CJS[pc]

            eye_sb = cpool.tile([128, 128], FP32R)
            nc.sync.dma_start(out=eye_sb[:, :], in_=eye[:, :])
            a_sb = cpool.tile([128, N], FP32)
            nc.sync.dma_start(out=a_sb[:, :], in_=a[:, :])
            d_sb = cpool.tile([128, F], FP32)
            nc.sync.dma_start(out=d_sb[:, :], in_=d[:, :])

            # PE warm-up burst during the DMA ramp (output unused).
            warm = wmpool.tile([128, 512], FP32, name="warm", tag="warm")
            for _ in range(10):
                nc.tensor.matmul(
                    warm[:, 0:128], eye_sb[:, :], eye_sb[:, :],
                    start=True, stop=True,
                )

            wsum = acpool.tile([128, JSLOTS * F], FP32)
            a_step = a_sb.ap[0][0]

            j0 = 0
            for c, CJ in enumerate(CJS):
                CJF = CJ * F
                if c < 2:
                    xt = xt_pre[c]
                else:
                    xt = xpool.tile([128, 3 * CJF], FP32R, name="xt", tag="xt")
                    for k in range(3):
                        src = bass.AP(
                            xk.tensor,
                            k * NH * N * F + j0 * F,
                            [[N * F, 128], [1, CJF]],
                        )
                        nc.sync.dma_start(
                            out=xt[:, k * CJF : (k + 1) * CJF], in_=src
                        )

                nslab = CJF // 512
                offl = nslab > 1   # slab 0 hop-summed on DVE, rest on PE
                if offl:
                    # Separate fp32 output tile: writing into xt itself
                    # trips the BIR verifier (fp32r-consumed location
                    # written by a non-fp32r-rounding producer).
                    hsum = hspool.tile([128, 512], FP32, name="hs", tag="hs")
                    nc.vector.tensor_add(
                        hsum[:, :],
                        xt[:, 0:512].bitcast(FP32),
                        xt[:, CJF : CJF + 512].bitcast(FP32),
                    )
                    nc.vector.tensor_add(
                        hsum[:, :],
                        hsum[:, :],
                        xt[:, 2 * CJF : 2 * CJF + 512].bitcast(FP32),
                    )
                pe_slabs = list(range(1, nslab)) if offl else [0]
                ps = pspool.tile(
                    [128, len(pe_slabs) * 512], FP32, name="ps", tag="ps"
                )
                for si, s in enumerate(pe_slabs):
                    for k in range(3):
                        nc.tensor.matmul(
                            ps[:, si * 512 : (si + 1) * 512],
                            eye_sb[:, :],
                            xt[:, k * CJF + s * 512 : k * CJF + (s + 1) * 512],
                            start=(k == 0),
                            stop=(k == 2),
                        )

                # prod[i, j*F+f] = hopsum[i, j*F+f] * a_sb[i, j0+j]
                prod = prpool.tile([128, CJF], FP32, name="prod", tag="prod")
                pr_step = prod.ap[0][0]
                ps_step = ps.ap[0][0]
                if offl:
                    po_a = bass.AP(
                        prod.tensor, 0, [[pr_step, 128], [F, 8], [1, F]]
                    )
                    in1a = bass.AP(
                        a_sb.tensor, j0, [[a_step, 128], [1, 8], [0, F]]
                    )
                    nc.vector.tensor_mul(po_a, hsum[:, :], in1a)
                    po_b = bass.AP(
                        prod.tensor, 512, [[pr_step, 128], [F, CJ - 8], [1, F]]
                    )
                    in0b = bass.AP(
                        ps.tensor, 0, [[ps_step, 128], [F, CJ - 8], [1, F]]
                    )
                    in1b = bass.AP(
                        a_sb.tensor, j0 + 8, [[a_step, 128], [1, CJ - 8], [0, F]]
                    )
                    nc.vector.tensor_mul(po_b, in0b, in1b)
                else:
                    po = bass.AP(
                        prod.tensor, 0, [[pr_step, 128], [F, CJ], [1, F]]
                    )
                    in0 = bass.AP(
                        ps.tensor, 0, [[ps_step, 128], [F, CJ], [1, F]]
                    )
                    in1 = bass.AP(
                        a_sb.tensor, j0, [[a_step, 128], [1, CJ], [0, F]]
                    )
                    nc.vector.tensor_mul(po, in0, in1)

                # Accumulate into the 16 j-slot accumulator.
                if c == 0:
                    # CJ=32: fold the two 1024-el halves straight into wsum,
                    # and pre-add the hop-0 diagonal here (DVE is idle this
                    # early) so the tail doesn't pay for it.
                    nc.vector.tensor_add(
                        wsum[:, :], prod[:, :1024], prod[:, 1024:2048]
                    )
                    nc.vector.tensor_add(
                        wsum[:, :F], wsum[:, :F], d_sb[:, :]
                    )
                else:
                    if CJF == 2048:
                        nc.vector.tensor_add(
                            prod[:, :1024], prod[:, :1024], prod[:, 1024:2048]
                        )
                    w = min(CJF, 1024)
                    nc.vector.tensor_add(
                        wsum[:, :w], wsum[:, :w], prod[:, :w]
                    )
                j0 += CJ
                if c == 7:
                    # Slots 8..15 are final once the 16-j chunk is in (the
                    # 8-j tail chunks only touch 0..7); fold them now,
                    # overlapped with the tail DMAs.
                    nc.vector.tensor_add(
                        wsum[:, : 8 * F],
                        wsum[:, : 8 * F],
                        wsum[:, 8 * F : 16 * F],
                    )

            # Collapse the remaining 8 j slots (hop-0 diagonal was already
            # added under chunk 0) and ship the result straight from wsum.
            h = JSLOTS // 4
            while h >= 1:
                nc.vector.tensor_add(
                    wsum[:, : h * F],
                    wsum[:, : h * F],
                    wsum[:, h * F : 2 * h * F],
                )
                h //= 2
            nc.sync.dma_start(out=out[:, :], in_=wsum[:, :F])

    nc.compile()
    _CACHE["nc"] = nc
    return nc


def _make_in_maps(A, X):
    idx = np.arange(NH)
    in_maps = []
    for c in range(8):
        b, h = c // 2, c % 2
        lo = h * NH
        xk = np.ascontiguousarray(X[b, 1:4, lo : lo + NH])
        av = np.ascontiguousarray(A[b, lo : lo + NH, :])
        dv = np.ascontiguousarray(X[b, 0, lo + idx, lo + idx, :])
        in_maps.append(
            {"xk": xk, "a": av, "d": dv, "eye": np.eye(128, dtype=np.float32)}
        )
    return in_maps


def run(A, X, trace=False, **kw):
    nc = _build_nc()
    in_maps = _make_in_maps(A, X)
    res = run_bass_kernel_spmd(
        nc, in_maps, core_ids=list(range(8)), trace=trace, **kw
    )
    out = np.empty((BATCH, N, F), dtype=np.float32)
    for c in range(8):
        b, h = c // 2, c % 2
        out[b, h * NH : (h + 1) * NH] = res.results[c]["out"]
    return out, res


def kernel(A, X):
    A = np.asarray(A, dtype=np.float32)
    X = np.asarray(X, dtype=np.float32)
    out, _ = run(A, X, trace=False)
    return out
